# revision 10
# baseline (speedup 1.0000x reference)
"""Trainium2 Bass kernel: single-head causal attention (B=16, T=2048, C=1024, HD=64).

Data-parallel over batch across 8 NeuronCores (2 batches/core), weights
replicated. Each core computes, per batch:
    q = x @ Wq, k = x @ Wk, v = x @ Wv            (via transposed layouts)
    scores[t, s] = k[t] . q[s] / sqrt(C)          (computed transposed: St[s, t])
    causal mask (keep s <= t), softmax over s, out[t] = sum_s w[t, s] v[s]

v3 design notes (informed by HW traces of v1/v2):
  - x loads are HWDGE fp32 (4KB descriptors, ~near HBM line rate). The
    fp32->bf16 cast is split per chunk between DVE (3 slabs) and GPSIMD
    (1 slab) so neither engine is critical. SWDGE cast-DMA was tried and
    rejected: the tile framework serializes SWDGE DMAs against in-flight
    DMA-xbar transposes (HW deadlock guard), which strangles the pipeline.
  - xT comes from one DMA-xbar transpose call per 512-t chunk. PE
    transpose-mode (is_transpose) was measured at ~470ns per 128x128 tile on
    HW (cost model says 53ns; believe the HW), so bulk x transposition stays
    on the xbar.
  - v [64h, s] and the finale ut [65, t] are transposed with NORMAL-mode
    matmuls (data tile as the stationary operand, identity moving): measured
    ~81-130ns per tile, output fp32 in PSUM, copied/cast out by DVE. This
    removes the vaug xbar call and the fp32 identity matmuls of v1.
  - exp runs on ACT over PAIRS of score tiles: each pair lands in one
    [128, 2, 512] fp32 PSUM tile (2 banks) and one ACTIVATE covers both
    halves when both are full-width, halving the 352-cycle per-call ACT
    overhead; diagonal pairs get per-tile narrowed calls (the race detector
    forbids exp of never-written PSUM lanes).
  - B-phase: single stream per chunk with one-pair lookahead: score matmuls
    of pair p+1 are enqueued before AV of pair p, so the in-order PE queue
    works on scores while exp(p) resolves on ACT; A-phase work of chunk k+1
    and the x-load of chunk k+2 are woven between B-steps of chunk k.
  - PSUM budget (8 banks): stq pairs 2x2 + qkps 1 + vps 1 + utps 1 + tp 1.
"""

import numpy as np

import concourse.bass as bass
import concourse.tile as tile
from concourse import bacc, mybir
from concourse.bass_utils import run_bass_kernel_spmd
from concourse.masks import make_identity

F32 = mybir.dt.float32
BF16 = mybir.dt.bfloat16

B, T, C, HD = 16, 2048, 1024, 64
N_CORES = 8
BL = B // N_CORES  # batches per core

P = 128
TCH = 512  # t-chunk (PSUM bank width in fp32)
VP = 72    # vaug padded inner dim


def build_attention(ctx, tc, out, x, wk, wq, wv, b_l, t_dim, c_dim):
    nc = tc.nc
    ncc = c_dim // P        # c chunks (contraction)
    nj = t_dim // TCH       # t chunks per batch
    ntt = TCH // P          # t subtiles per chunk
    nst = t_dim // P        # s tiles
    nk = b_l * nj           # global chunks
    scale = 1.0 / float(np.sqrt(np.float32(c_dim)))

    const_pool = ctx.enter_context(tc.tile_pool(name="const", bufs=1))
    io_pool = ctx.enter_context(tc.tile_pool(name="io", bufs=1))
    big_pool = ctx.enter_context(tc.tile_pool(name="big", bufs=1))
    per_b = ctx.enter_context(tc.tile_pool(name="per_b", bufs=1))
    work = ctx.enter_context(tc.tile_pool(name="work", bufs=1))
    ps = ctx.enter_context(tc.tile_pool(name="ps", bufs=1, space="PSUM"))

    identb = const_pool.tile([P, P], BF16, name="identb")
    wq_bf = const_pool.tile([P, ncc, HD], BF16, name="wq_bf")
    wk_bf = const_pool.tile([P, ncc, HD], BF16, name="wk_bf")
    wqk_sb = const_pool.tile([P, ncc, 2 * HD], BF16, name="wqk_sb")
    wv_sb = const_pool.tile([P, ncc, HD], BF16, name="wv_sb")

    masks = {}

    def const_setup():
        # identity + causal mask first: everything on the gpsimd queue ahead
        # of them delays the PE warmup
        make_identity(nc, identb)
        mk = const_pool.tile([P, TCH], BF16, name="mask_diag")
        nc.gpsimd.memset(mk[:], 1.0)
        nc.gpsimd.affine_select(
            out=mk,
            in_=mk,
            compare_op=mybir.AluOpType.is_ge,
            fill=0.0,
            base=0,
            channel_multiplier=-1,
            pattern=[[1, TCH]],
        )
        masks[0] = mk

    def weights_setup():
        # Stationary weights via SWDGE cast-DMA (fp32 -> bf16 inline):
        # ~1.3us issue each on the otherwise idle gpsimd queue, vs 4-10us
        # HWDGE issue for the same strided-dst pattern. They complete before
        # the first xbar transpose, so the SWDGE-vs-transpose serialization
        # guard never engages.
        nc.gpsimd.dma_start(wq_bf, wq.rearrange("(o p) h -> p o h", p=P))
        nc.gpsimd.dma_start(wk_bf, wk.rearrange("(o p) h -> p o h", p=P))
        nc.gpsimd.dma_start(wv_sb, wv.rearrange("(o p) h -> p o h", p=P))
        nc.vector.tensor_copy(wqk_sb[:, :, 0:HD], wq_bf)
        nc.vector.tensor_copy(wqk_sb[:, :, HD : 2 * HD], wk_bf)

    def warmup():
        # Warm the PE (HAM clock gate) during the DMA-bound startup and give
        # the stq/utps PSUM banks bounded initial values.
        for w in range(2):
            stq = ps.tile([P, 2, TCH], F32, tag="stq", bufs=2, name=f"warm_{w}")
            for d in range(2):
                for g in range(ntt):
                    nc.tensor.matmul(
                        stq[:, d, g * P : (g + 1) * P], identb, identb,
                        start=True, stop=True,
                    )
        ut = ps.tile([HD + 1, TCH], F32, tag="utps", bufs=1, name="warm_ut")
        for g in range(ntt):
            nc.tensor.matmul(
                ut[:, g * P : (g + 1) * P], identb[:, 0 : HD + 1], identb,
                start=True, stop=True,
            )

    qkT = {}   # q rows 0:64, k rows 64:128
    qkT2 = {}  # k rows 0:64, q rows 64:128
    vaug = {}

    def batch_setup(b):
        qkT[b] = per_b.tile([P, t_dim], BF16, name=f"qkT_{b}", tag="qkT", bufs=2)
        qkT2[b] = per_b.tile([P, t_dim], BF16, name=f"qkT2_{b}", tag="qkT2", bufs=2)
        vaug[b] = per_b.tile([P, nst, VP], BF16, name=f"vaug_{b}", tag="vaug", bufs=2)
        nc.gpsimd.memset(vaug[b][:, :, HD], 1.0)

    def a_phase_thunks(k, xbar_sync):
        """Load one 512-t chunk of x, cast to bf16 (DVE+GPSIMD), xbar
        transpose, project q/k/v, build qkT/qkT2/vaug entries."""
        b, j = divmod(k, nj)
        st = {}
        jt = slice(j * TCH, (j + 1) * TCH)

        def load():
            xin = io_pool.tile(
                [P, ntt, c_dim], F32, tag="xin", bufs=4, name=f"xin_{k}"
            )
            nc.scalar.dma_start(
                xin, x[b, jt, :].rearrange("(tt p) c -> p tt c", p=P)
            )
            st["xin"] = xin

        def cast():
            xbf = io_pool.tile(
                [P, ntt, c_dim], BF16, tag="xbf", bufs=4, name=f"xbf_{k}"
            )
            # split the fp32->bf16 cast: DVE does 7/8, GPSIMD 1/8
            ch = c_dim // 2
            nc.vector.tensor_copy(xbf[:, 0 : ntt - 1, :], st["xin"][:, 0 : ntt - 1, :])
            nc.vector.tensor_copy(xbf[:, ntt - 1, 0:ch], st["xin"][:, ntt - 1, 0:ch])
            nc.gpsimd.tensor_copy(xbf[:, ntt - 1, ch:c_dim], st["xin"][:, ntt - 1, ch:c_dim])
            st["xbf"] = xbf

        def trans():
            st["xT"] = big_pool.tile(
                [P, ntt * ncc, P], BF16, tag="xT", bufs=4, name=f"xT_{k}"
            )
            # all xbar transposes on ONE queue: two concurrent transposes
            # (issued from different HWDGE queues) corrupted data on HW
            nc.sync.dma_start_transpose(st["xT"], st["xbf"])

        def make_proj(cc0, cc1):
            def f():
                if cc0 == 0:
                    st["qkps"] = ps.tile(
                        [P, TCH], F32, tag="qkps", bufs=1, name=f"qkps_{k}"
                    )
                    st["vps"] = ps.tile(
                        [HD, TCH], F32, tag="vps", bufs=1, name=f"vps_{k}"
                    )
                for cc in range(cc0, cc1):
                    # [128, tt, 128] strided view: columns t = tt*128 + t_lo
                    rhs = st["xT"][:, cc : ntt * ncc : ncc, :]
                    nc.tensor.matmul(
                        st["qkps"],
                        wqk_sb[:, cc, :],
                        rhs,
                        start=(cc == 0),
                        stop=(cc == ncc - 1),
                    )
                    nc.tensor.matmul(
                        st["vps"],
                        wv_sb[:, cc, :],
                        rhs,
                        start=(cc == 0),
                        stop=(cc == ncc - 1),
                    )
            return f

        def epilogue():
            nc.vector.tensor_copy(qkT[b][:, jt], st["qkps"])
            nc.vector.tensor_copy(qkT2[b][0:HD, jt], st["qkps"][HD:P, :])
            nc.vector.tensor_copy(qkT2[b][HD:P, jt], st["qkps"][0:HD, :])
            vtsb = work.tile([HD, TCH], BF16, tag="vtsb", bufs=2, name=f"vt_{k}")
            nc.vector.tensor_copy(vtsb, st["vps"])
            # PE-transpose v [64h, t] -> [t, 64h] (normal-mode: v tile is the
            # stationary operand, identity moves; ~100ns/tile)
            vtp = ps.tile([P, ntt, P], F32, tag="tp", bufs=1, name=f"vtp_{k}")
            for g in range(ntt):
                nc.tensor.matmul(
                    vtp[:, g, 0:HD],
                    vtsb[:, g * P : (g + 1) * P],
                    identb[0:HD, 0:HD],
                    start=True,
                    stop=True,
                )
            nc.vector.tensor_copy(
                vaug[b][:, j * ntt : (j + 1) * ntt, 0:HD], vtp[:, :, 0:HD]
            )

        return [load, cast, trans, make_proj(0, (ncc + 1) // 2),
                make_proj((ncc + 1) // 2, ncc), epilogue]

    def b_phase_thunks(k):
        """Attention for one 512-t chunk: per s-tile pair score/exp/mask, AV
        with one-pair lag, then the finale (transpose back, divide, store)."""
        b, j = divmod(k, nj)
        st = {}
        ni = ntt * (j + 1)
        npair = ni // 2
        jt0 = j * TCH

        def tile_geom(i):
            diag = i >= ntt * j
            w0 = (i - ntt * j) * P if diag else 0
            return diag, w0, TCH - w0

        def make_scores(p):
            def f():
                if p == 0:
                    st["utps"] = ps.tile(
                        [HD + 1, TCH], F32, tag="utps", bufs=1, name=f"ut_{k}"
                    )
                stq = ps.tile(
                    [P, 2, TCH], F32, tag="stq", bufs=2, name=f"st_{k}_{p}"
                )
                for d in range(2):
                    i = 2 * p + d
                    diag, w0, wn = tile_geom(i)
                    lo, hi = (0, HD) if i % 2 == 0 else (HD, P)
                    lhs = qkT[b] if i % 2 == 0 else qkT2[b]
                    rhs = qkT2[b] if i % 2 == 0 else qkT[b]
                    nc.tensor.matmul(
                        stq[:, d, w0:TCH],
                        lhs[lo:hi, i * P : (i + 1) * P],
                        rhs[lo:hi, jt0 + w0 : jt0 + TCH],
                        start=True,
                        stop=True,
                    )
                pt = work.tile(
                    [P, 2, TCH], BF16, tag="pt", bufs=3, name=f"pt_{k}_{p}"
                )
                st[("pt", p)] = pt
                if all(tile_geom(2 * p + d)[1] == 0 for d in range(2)):
                    # both tiles full-width: one fused ACTIVATE over 2 banks
                    nc.scalar.activation(
                        pt, stq, mybir.ActivationFunctionType.Exp, scale=scale
                    )
                else:
                    for d in range(2):
                        w0 = tile_geom(2 * p + d)[1]
                        nc.scalar.activation(
                            pt[:, d, w0:TCH], stq[:, d, w0:TCH],
                            mybir.ActivationFunctionType.Exp, scale=scale,
                        )
                for d in range(2):
                    i = 2 * p + d
                    diag, w0, wn = tile_geom(i)
                    if diag:
                        nc.vector.tensor_mul(
                            pt[:, d, w0:TCH], pt[:, d, w0:TCH], masks[0][:, 0:wn]
                        )
            return f

        def make_av(p):
            def f():
                pt = st[("pt", p)]
                for d in range(2):
                    i = 2 * p + d
                    diag, w0, wn = tile_geom(i)
                    nc.tensor.matmul(
                        st["utps"][:, w0:TCH],
                        vaug[b][:, i, 0 : HD + 1],
                        pt[:, d, w0:TCH],
                        start=(i == 0),
                        stop=(i == ni - 1),
                        skip_group_check=True,
                    )
            return f

        def finale():
            utsb = work.tile(
                [HD + 1, TCH], BF16, tag="utsb", bufs=2, name=f"utsb_{k}"
            )
            nc.vector.tensor_copy(utsb, st["utps"])
            # normal-mode PE transpose back with a j-strided stationary:
            # otp[p, g, 0:65] = ut[:, p*ntt+g], so partition p holds ntt
            # consecutive t rows -> 1KB-contiguous store descriptors
            otp = ps.tile([P, ntt, P], F32, tag="tp", bufs=1, name=f"otp_{k}")
            for g in range(ntt):
                nc.tensor.matmul(
                    otp[:, g, 0 : HD + 1],
                    utsb[:, g : TCH : ntt],
                    identb[0 : HD + 1, 0 : HD + 1],
                    start=True,
                    stop=True,
                )
            rec = work.tile([P, ntt], F32, tag="rec", bufs=2, name=f"rec_{k}")
            nc.vector.reciprocal(rec, otp[:, :, HD])
            osb = io_pool.tile(
                [P, ntt, HD], F32, tag="osb", bufs=2, name=f"osb_{k}"
            )
            nc.vector.tensor_mul(
                osb, otp[:, :, 0:HD], rec.broadcast_to([P, ntt, HD])
            )
            nc.scalar.dma_start(
                out[b, jt0 : jt0 + TCH, :].rearrange("(p q) h -> p q h", p=P),
                osb,
            )

        sc = [make_scores(p) for p in range(npair)]
        av = [make_av(p) for p in range(npair)]
        lst = [sc[0]]
        for p in range(1, npair):
            lst += [sc[p], av[p - 1]]
        lst += [av[npair - 1], finale]
        return lst

    def weave_list(stream, inject):
        units = []
        nsteps = len(stream)
        ai, na = 0, len(inject)
        for step in range(nsteps):
            u = [stream[step]]
            tgt = (step + 1) * na // nsteps
            while ai < tgt:
                u.append(inject[ai])
                ai += 1
            units.append(u)
        return units

    # chunk processing order interleaves the two batches so the heavy
    # late-j B-phases don't all stack up in the tail
    seq = [b * nj + j for j in range(nj) for b in range(b_l)]
    A = {k: a_phase_thunks(k, m % 2 == 0) for m, k in enumerate(seq)}

    # startup: identity/mask first (gpsimd queue), x loads out 4 deep,
    # weights, PE warmup immediately, then A of the first chunk
    const_setup()
    for m in range(min(4, nk)):
        A[seq[m]][0]()
    weights_setup()
    warmup()
    for b in range(b_l):
        batch_setup(b)
    for t in A[seq[0]][1:]:
        t()

    all_units = []
    for m in range(nk):
        inject = []
        if m + 4 < nk:
            inject.append(A[seq[m + 4]][0])
        if m + 1 < nk:
            inject += A[seq[m + 1]][1:]
        all_units += weave_list(b_phase_thunks(seq[m]), inject)
    for u in all_units:
        for t in u:
            t()


def build_nc(b_l=BL, t_dim=T, c_dim=C):
    nc = bacc.Bacc("TRN2", target_bir_lowering=False, debug=False)
    x = nc.dram_tensor("x", [b_l, t_dim, c_dim], F32, kind="ExternalInput").ap()
    wk = nc.dram_tensor("Wk", [c_dim, HD], F32, kind="ExternalInput").ap()
    wq = nc.dram_tensor("Wq", [c_dim, HD], F32, kind="ExternalInput").ap()
    wv = nc.dram_tensor("Wv", [c_dim, HD], F32, kind="ExternalInput").ap()
    out = nc.dram_tensor("out", [b_l, t_dim, HD], F32, kind="ExternalOutput").ap()
    from contextlib import ExitStack

    with tile.TileContext(nc) as tc, ExitStack() as ctx:
        build_attention(ctx, tc, out, x, wk, wq, wv, b_l, t_dim, c_dim)
    nc.compile()
    return nc


_NC_CACHE = {}


def _get_nc():
    if "nc" not in _NC_CACHE:
        _NC_CACHE["nc"] = build_nc()
    return _NC_CACHE["nc"]


def kernel(x, Wk, Wq, Wv, _trace=False, _tmpdir=None):
    x = np.ascontiguousarray(np.asarray(x, dtype=np.float32))
    Wk = np.ascontiguousarray(np.asarray(Wk, dtype=np.float32))
    Wq = np.ascontiguousarray(np.asarray(Wq, dtype=np.float32))
    Wv = np.ascontiguousarray(np.asarray(Wv, dtype=np.float32))
    nc = _get_nc()
    in_maps = [
        {"x": x[c * BL : (c + 1) * BL], "Wk": Wk, "Wq": Wq, "Wv": Wv}
        for c in range(N_CORES)
    ]
    res = run_bass_kernel_spmd(
        nc, in_maps, core_ids=list(range(N_CORES)), trace=_trace, tmpdir=_tmpdir
    )
    out = np.concatenate([res.results[c]["out"] for c in range(N_CORES)], axis=0)
    if _trace:
        kernel.last_exec_time_ns = res.exec_time_ns
        kernel.last_results = res
    return out
